# revision 30
# baseline (speedup 1.0000x reference)
"""Trainium2 Bass kernel: MultiHeadCrossAttentionWithBias.

Reference computation (per batch b):
  q_u = scale*(u_enc @ wq + wq_b); k/v from e_enc (and vice versa)
  ue_w = softmax(q_u k_e^T + bpp + mask*-inf); u_ctx = ue_w @ v_e
  u_update = u_ctx @ wo + wo_b                     (same mirrored for e)

Sharding: the problem decomposes into 8 fully independent attention units:
(batch b, direction d) for b in 0..3, d in {u->e, e->u}. Core i = (d, b)
handles one unit end-to-end; no collectives needed.

Host prep is layout/precision only (transposes, slices, fp32->bf16
rounding of matmul operands); all FLOPs run on device.

Per-core inputs:
  encQT  [D=512, L=1024] bf16  query-side encoder, transposed
  encKT  [D=512, L=1024] bf16  key-side encoder, transposed
  bpp    [L, L] bf16           logit bias oriented [k, q]
  mask   [L, L] uint8          mask oriented [k, q]
  wq/wk/wv [D, 512] bf16, wo [512, D] bf16, biases f32

On-device math (per core), all matmul operands bf16 (FWL-friendly, no
fp32 slow paths; PSUM accumulation stays f32):
  qT[f, s] = scale*(wq^T encQT + wq_b)   (f = h*64+hd on partitions)
  kT[f, s] =        wk^T encKT + wk_b
  v[s, f]  =        encKT^T wv + wv_b    (+ fused ones column per head)
  CB[k, q] = bpp_w*bpp + bpp_b + (mask-1)*1e30   (DVE+gpsimd, -> bf16)
  per head h, k-chunk kc:
      S^T = CB[kc]                 (PE: identity-stationary matmul, start)
      S^T += kT_h^T qT_h           (PE accumulation, stop)
      E = exp(S^T)                 (ACT; no max-subtraction: logits O(10))
      [ctx^T; den] += [v_h | 1]^T E  (PE, PSUM accumulation over kc)
  The CB injection rides the PE (instead of a DVE add on the critical
  path) so the tensor engine stream stays dense: PE-HAM then holds the
  2.4 GHz clock state through the attention phase.
  rcp = approx_reciprocal(den); partition-broadcast via PE matmul with a
  2-row selector stationary (no DRAM bounce)
  ctxn[pair] = ctx^T * rcp  (DVE, odd head written to partitions 64..127)
  out[s, e] = sum_pair ctxn_p^T wo_p + wo_b   (PE + DVE bias-add eviction)
"""

import numpy as np
from contextlib import ExitStack

import ml_dtypes

import concourse.bass as bass
import concourse.tile as tile
import concourse.bacc as bacc
import concourse.mybir as mybir
from concourse.masks import make_identity
from concourse import bass_utils

F32 = mybir.dt.float32
U8 = mybir.dt.uint8
BF16 = mybir.dt.bfloat16
AF = mybir.ActivationFunctionType
ALU = mybir.AluOpType

B, L, D, H, HD = 4, 1024, 512, 8, 64
P = 128
FH = H * HD            # 512
SCALE = 1.0 / np.sqrt(HD)
NEG = -1.0e30
N_CORES = 8


def bcast_ap(dram_ap, parts):
    """Partition-step-0 broadcast AP over a DRAM row."""
    return bass.AP(tensor=dram_ap.tensor, offset=dram_ap.offset,
                   ap=[[0, parts]] + list(dram_ap.ap))


def build_module():
    nc = bacc.Bacc("TRN2", target_bir_lowering=False, debug=False)

    # inputs packed on host to [128, n*C]: row-chunk c of the logical
    # tensor sits at columns [c*C, (c+1)*C) -> one or two DMA
    # descriptors per tensor instead of one per 128-row chunk
    encQT_d = nc.dram_tensor("encQT", [P, 4 * L], BF16, kind="ExternalInput")
    encKT_d = nc.dram_tensor("encKT", [P, 4 * L], BF16, kind="ExternalInput")
    wq_d = nc.dram_tensor("wq", [P, 4 * FH], BF16, kind="ExternalInput")
    wk_d = nc.dram_tensor("wk", [P, 4 * FH], BF16, kind="ExternalInput")
    wv_d = nc.dram_tensor("wv", [P, 4 * FH], BF16, kind="ExternalInput")
    wo_d = nc.dram_tensor("wo", [P, 4 * D], BF16, kind="ExternalInput")
    bpp_d = nc.dram_tensor("bpp", [P, 8 * L], BF16, kind="ExternalInput")
    mask_d = nc.dram_tensor("mask", [P, 8 * L], U8, kind="ExternalInput")
    wqb_d = nc.dram_tensor("wqb", [FH], F32, kind="ExternalInput")
    wkb_d = nc.dram_tensor("wkb", [FH], F32, kind="ExternalInput")
    wvb_d = nc.dram_tensor("wvb", [FH], F32, kind="ExternalInput")
    wob_d = nc.dram_tensor("wob", [D], F32, kind="ExternalInput")
    bppw_d = nc.dram_tensor("bppw", [1, 1], F32, kind="ExternalInput")
    bppb_d = nc.dram_tensor("bppb", [1, 1], F32, kind="ExternalInput")
    out_d = nc.dram_tensor("out", [L, D], F32, kind="ExternalOutput")

    with tile.TileContext(nc) as tc, ExitStack() as ctx:
        const = ctx.enter_context(tc.tile_pool(name="const", bufs=1))
        qkT_p = ctx.enter_context(tc.tile_pool(name="qkT", bufs=8))
        v_p = ctx.enter_context(tc.tile_pool(name="v", bufs=8))
        wo_p = ctx.enter_context(tc.tile_pool(name="wo", bufs=1))
        cb_p = ctx.enter_context(tc.tile_pool(name="cb", bufs=8))
        ps_s = tc.alloc_tile_pool(name="ps_s", bufs=2, space="PSUM")
        ps_c = tc.alloc_tile_pool(name="ps_c", bufs=4, space="PSUM")

        # ---- small bias prep (tiny DMAs) ----
        # bpp_w / bpp_b broadcast to [128,1] columns
        bw_col = const.tile([P, 1], F32)
        nc.gpsimd.dma_start(bw_col[:], bcast_ap(bppw_d.ap()[0:1, :], P))
        bb_col = const.tile([P, 1], F32)
        nc.gpsimd.dma_start(bb_col[:], bcast_ap(bppb_d.ap()[0:1, :], P))
        # projection biases
        wqb_raw = const.tile([P, 4], F32)
        nc.gpsimd.dma_start(wqb_raw[:], wqb_d.ap().rearrange("(c p) -> p c", p=P))
        wqb_sc = const.tile([P, 4], F32)
        nc.vector.tensor_scalar_mul(wqb_sc[:], wqb_raw[:], float(SCALE))
        wkb_c = const.tile([P, 4], F32)
        nc.gpsimd.dma_start(wkb_c[:], wkb_d.ap().rearrange("(c p) -> p c", p=P))
        wvb_bc = const.tile([P, FH], F32)
        nc.gpsimd.dma_start(wvb_bc[:], bcast_ap(wvb_d.ap(), P))
        wob_bc = const.tile([P, D], F32)
        nc.gpsimd.dma_start(wob_bc[:], bcast_ap(wob_d.ap(), P))

        # selector for the denominator partition-broadcast (engine writes
        # must start at partition 0/32/64/96, so the two live rows sit at
        # partitions 0 and 32): sel[0, 0:64] = 1, sel[32, 64:128] = 1.
        # f32r keeps the denominator at full precision through the PE.
        F32R = mybir.dt.float32r
        sel = const.tile([33, P], F32R)
        nc.gpsimd.memset(sel[:].bitcast(F32), 0.0)
        nc.gpsimd.memset(sel[0:1, 0:HD].bitcast(F32), 1.0)
        nc.gpsimd.memset(sel[32:33, HD:P].bitcast(F32), 1.0)

        # ---- bias factor ECB[k, q] = exp(bpp*w + b + (mask-1)*1e30) ----
        # softmax bias applied multiplicatively: exp(S + CB) =
        # exp(S) * exp(CB), with exp(CB) precomputed overlapped with the
        # projections. Masked entries become exact 0, so the post-softmax
        # re-mask is free. (m*1e30) + (-1e30) is exact for m in {0,1}.
        # The mask/bpp DMAs ride the gpsimd queue concurrently with the
        # sync-queue weight/encoder loads, landing in a deep dedicated
        # pool so the shared DMA engines never block on tile reuse. The
        # compute (DVE scale ops, pool add, ACT exp) is emitted inside
        # the q-projection loop, two k-chunks per pc, so no engine sees
        # a burst right when attention starts.
        cb = []
        cbd_p = tc.alloc_tile_pool(name="cbdma", bufs=1)
        cbt_p = tc.alloc_tile_pool(name="cbtmp", bufs=3)
        mask_tl = cbd_p.tile([P, 8 * L], U8, tag="m", name="mask")
        bpp_tl = cbd_p.tile([P, 8 * L], BF16, tag="b", name="bpp")

        def emit_cb_dmas():
            for i in range(4):
                s = slice(i * 2 * L, (i + 1) * 2 * L)
                nc.gpsimd.dma_start(mask_tl[:, s], mask_d.ap()[:, s])
            for i in range(8):
                s = slice(i * L, (i + 1) * L)
                nc.gpsimd.dma_start(bpp_tl[:, s], bpp_d.ap()[:, s])

        def emit_cb_compute(kc):
            mn_t = cbt_p.tile([P, L], BF16, tag="mn", name=f"mn{kc}")
            nc.vector.tensor_scalar(mn_t[:],
                                    mask_tl[:, kc * L:(kc + 1) * L], -NEG,
                                    NEG, ALU.mult, ALU.add)
            bs_t = cbt_p.tile([P, L], BF16, tag="bs", name=f"bs{kc}")
            nc.vector.tensor_scalar(bs_t[:],
                                    bpp_tl[:, kc * L:(kc + 1) * L],
                                    bw_col[:, 0:1], bb_col[:, 0:1],
                                    ALU.mult, ALU.add)
            c_t = cbt_p.tile([P, L], F32, tag="cb", name=f"cb{kc}")
            nc.gpsimd.tensor_add(c_t[:], mn_t[:], bs_t[:])
            e_t = cb_p.tile([P, L], BF16, tag="ecb", name=f"ecb{kc}")
            nc.scalar.activation(e_t[:], c_t[:], AF.Exp)
            cb.append(e_t)

        # ---- projections ----
        qT, kT, v_aug = [], [], []
        with tc.tile_pool(name="enc", bufs=1) as enc_p, \
             tc.tile_pool(name="wqkv", bufs=1) as w_p:
            # ~4 descriptors per tensor: parallel across DMA engines
            # but cheap to issue; weight and encoder issue ride separate
            # queues (sync / vector) so neither serializes the other
            def loadn(pool, eng, dram, cols, n, tag, dt=BF16):
                t = pool.tile([P, cols], dt, tag=tag,
                              name=f"t_{dram.name}")
                step = cols // n
                for i in range(n):
                    eng.dma_start(t[:, i * step:(i + 1) * step],
                                  dram.ap()[:, i * step:(i + 1) * step])
                return t

            wq_tl = loadn(w_p, nc.sync, wq_d, 4 * FH, 4, "wq")
            eq_tl = loadn(enc_p, nc.scalar, encQT_d, 4 * L, 4, "eq")
            emit_cb_dmas()
            wk_tl = loadn(w_p, nc.sync, wk_d, 4 * FH, 4, "wk")
            ek_tl = loadn(enc_p, nc.scalar, encKT_d, 4 * L, 4, "ek")
            wv_tl = loadn(w_p, nc.sync, wv_d, 4 * FH, 4, "wv")
            wq_t = [wq_tl[:, dc * FH:(dc + 1) * FH] for dc in range(4)]
            wk_t = [wk_tl[:, dc * FH:(dc + 1) * FH] for dc in range(4)]
            wv_t = [wv_tl[:, dc * FH:(dc + 1) * FH] for dc in range(4)]
            eq = [eq_tl[:, dc * L:(dc + 1) * L] for dc in range(4)]
            ek = [ek_tl[:, dc * L:(dc + 1) * L] for dc in range(4)]


            # qT / kT: [f, s] packed two heads per 128-partition chunk
            for which, w_tl, enc_tl, out_list in (
                ("q", wq_tl, eq_tl, qT), ("k", wk_tl, ek_tl, kT),
            ):
                for pc in range(4):
                    o = qkT_p.tile([P, L], BF16, tag="qkT",
                                   name=f"{which}T{pc}")
                    for sh in range(2):
                        ps = ps_s.tile([P, 512], F32, tag="ps_s",
                                       name=f"ps_{which}{pc}_{sh}")
                        for dc in range(4):
                            nc.tensor.matmul(
                                ps[:],
                                w_tl[:, dc * FH + pc * P:
                                     dc * FH + (pc + 1) * P],
                                enc_tl[:, dc * L + sh * 512:
                                       dc * L + (sh + 1) * 512],
                                start=(dc == 0), stop=(dc == 3))
                        sl = slice(sh * 512, (sh + 1) * 512)
                        # evict on DVE: the ACT queue stays free for the
                        # ecb exps + attention exps
                        if which == "q":
                            nc.vector.tensor_scalar(
                                o[:, sl], ps[:], float(SCALE),
                                wqb_sc[:, pc:pc + 1], ALU.mult, ALU.add)
                        else:
                            nc.vector.tensor_scalar(
                                o[:, sl], ps[:], wkb_c[:, pc:pc + 1], None,
                                ALU.add)
                    out_list.append(o)
                    emit_cb_compute(pc if which == "q" else 4 + pc)

            # v: [s, f] with ones column interleaved per head ([128, 8*65])
            for sc in range(8):
                ps = ps_s.tile([P, 512], F32, tag="ps_s", name=f"ps_v{sc}")
                for dc in range(4):
                    nc.tensor.matmul(
                        ps[:],
                        ek_tl[:, dc * L + sc * P:dc * L + sc * P + P],
                        wv_tl[:, dc * FH:(dc + 1) * FH],
                        start=(dc == 0), stop=(dc == 3))
                va = v_p.tile([P, H * (HD + 1)], BF16, tag="v", name=f"v{sc}")
                vg = va[:].rearrange("p (h c) -> p h c", c=HD + 1)
                nc.vector.scalar_tensor_tensor(
                    vg[:, :, 0:HD],
                    ps[:].rearrange("p (h c) -> p h c", c=HD), 1.0,
                    wvb_bc[:].rearrange("p (h c) -> p h c", c=HD),
                    ALU.bypass, ALU.add)
                nc.vector.memset(vg[:, :, HD:HD + 1], 1.0)
                v_aug.append(va)

        cbt_p.release()
        cbd_p.release()

        # ---- wo load (late: not projection-critical) ----
        wo_tl = wo_p.tile([P, 4 * D], BF16, tag="wo", name="wo")
        for i in range(4):
            s = slice(i * D, (i + 1) * D)
            nc.gpsimd.dma_start(wo_tl[:, s], wo_d.ap()[:, s])

        # ---- attention (head pairs, PE row-tiled QK) ----
        # The two heads of a pair occupy partitions 0..63 / 64..127 of
        # qT/kT, so their K=64 QK matmuls land in different PE row
        # groups and execute concurrently (row tiling: ~2x QK
        # throughput). Per (pair, kc): 4 QK matmuls in ~2 matmul-times
        # -> exp per head (ACT) -> bf16 multiply by exp(CB) (DVE) ->
        # lagged PV (full-K contraction, serial). The pair softmax
        # denominators are broadcast via a DRAM bounce (no PSUM), with
        # the DVE/pool work spread one stage per kc of the next pair;
        # the last pair broadcasts through the PE into freed PSUM.
        ctxn_p = ctx.enter_context(tc.tile_pool(name="ctxn", bufs=4))
        ctxr_p = ctx.enter_context(tc.tile_pool(name="ctxr", bufs=2))
        den_p = ctx.enter_context(tc.tile_pool(name="den", bufs=3))
        den_d = nc.dram_tensor("den_scratch", [H, L], F32, kind="Internal")
        ctxn = [None] * 4

        def make_norm_stages(pc, ctxr, den_sb):
            # DRAM-bounce partition-broadcast of the raw denominators,
            # then reciprocal (DVE) + multiply (gpsimd), staged so no
            # engine sees a lump right when the next pair starts.
            rb = den_p.tile([P, L], F32, tag="rb", name=f"rb{pc}")
            rcp_sb = den_p.tile([P, L], F32, tag="rcp", name=f"rcp{pc}")

            def st_a():
                nc.sync.dma_start(den_d.ap()[2 * pc:2 * pc + 1, :],
                                  den_sb[0:1, :].bitcast(F32))
                nc.sync.dma_start(den_d.ap()[2 * pc + 1:2 * pc + 2, :],
                                  den_sb[32:33, :].bitcast(F32))
                nc.gpsimd.dma_start(
                    rb[0:HD, :], bcast_ap(den_d.ap()[2 * pc:2 * pc + 1, :], HD))
                nc.gpsimd.dma_start(
                    rb[HD:P, :],
                    bcast_ap(den_d.ap()[2 * pc + 1:2 * pc + 2, :], HD))

            def st_rcp(qh):
                sl = slice(qh * 512, (qh + 1) * 512)
                nc.vector.reciprocal(rcp_sb[:, sl], rb[:, sl])

            def st_mul(qh):
                sl = slice(qh * 512, (qh + 1) * 512)
                nc.gpsimd.tensor_mul(ctxn[pc][:, sl], ctxr[:, sl],
                                     rcp_sb[:, sl])

            return [st_a, lambda: st_rcp(0), lambda: st_mul(0),
                    lambda: st_rcp(1), lambda: st_mul(1)]

        def emit_norm_psum(pc, ctxr, den_sb):
            # last pair: PSUM is free, broadcast through the PE
            for qh in range(2):
                sl = slice(qh * 512, (qh + 1) * 512)
                rb_ps = ps_c.tile([P, 512], F32, tag="ps_c",
                                  name=f"rbl_{pc}_{qh}")
                nc.tensor.matmul(rb_ps[:], sel[:], den_sb[:, sl],
                                 start=True, stop=True)
                rcp_sb = den_p.tile([P, 512], F32, tag="rcpl",
                                    name=f"rcpl_{pc}_{qh}")
                for i in range(4):
                    nc.vector.reciprocal(rcp_sb[:, i * 128:(i + 1) * 128],
                                         rb_ps[:, i * 128:(i + 1) * 128])
                nc.gpsimd.tensor_mul(ctxn[pc][:, sl], ctxr[:, sl],
                                     rcp_sb[:])

        norm_stages = []
        with tc.tile_pool(name="e", bufs=14) as e_p, \
             tc.tile_pool(name="e0", bufs=4) as e0_p:
            for pc in range(4):
                LAG = 6 if pc == 0 else 3
                c_ps = {(hh, qh): ps_c.tile([HD + 1, 512], F32, tag="ps_c",
                                            name=f"c_ps_{pc}_{hh}_{qh}")
                        for hh in range(2) for qh in range(2)}
                e_ts = {}
                for kc in range(8 + LAG):
                    if kc < 8:
                        s_hh = [ps_s.tile([P, L], F32, tag="ps_s",
                                          name=f"s_ps_{pc}_{hh}_{kc}")
                                for hh in range(2)]
                        for qh in range(2):
                            sl = slice(qh * 512, (qh + 1) * 512)
                            for hh in range(2):
                                o = hh * HD
                                nc.tensor.matmul(
                                    s_hh[hh][:, sl],
                                    kT[pc][o:o + HD, kc * P:(kc + 1) * P],
                                    qT[pc][o:o + HD, sl],
                                    start=True, stop=True)
                        for hh in range(2):
                            e0 = e0_p.tile([P, L], BF16, tag="e0",
                                           name=f"e0_{pc}_{hh}_{kc}")
                            nc.scalar.activation(e0[:], s_hh[hh][:], AF.Exp)
                            et = e_p.tile([P, L], BF16, tag="e",
                                          name=f"e_{pc}_{hh}_{kc}")
                            nc.vector.tensor_mul(et[:], e0[:], cb[kc][:])
                            e_ts[(hh, kc)] = et
                    if norm_stages:
                        norm_stages.pop(0)()
                    if kc >= LAG:
                        kp = kc - LAG
                        for hh in range(2):
                            h = 2 * pc + hh
                            for qh in range(2):
                                sl = slice(qh * 512, (qh + 1) * 512)
                                nc.tensor.matmul(
                                    c_ps[(hh, qh)][:],
                                    v_aug[kp][:, h * (HD + 1):
                                              (h + 1) * (HD + 1)],
                                    e_ts[(hh, kp)][:, sl],
                                    start=(kp == 0), stop=(kp == 7))
                # evict raw ctx + denominator rows (ACT, odd head
                # partition-shifted to 64..127), freeing PSUM. For pairs
                # 0-2 the evictions are staged into the next pair's kc
                # loop so the ACT queue never lumps ahead of its exps.
                ctxn[pc] = ctxn_p.tile([P, L], BF16, tag="ctxn",
                                       name=f"ctxn{pc}")
                ctxr = ctxr_p.tile([P, L], F32, tag="ctxr",
                                   name=f"ctxr{pc}")
                den_sb = den_p.tile([33, L], mybir.dt.float32r,
                                    tag="den", name=f"den{pc}")
                # rows 1..31 are read (x0) by the last pair's broadcast
                nc.vector.memset(den_sb[:].bitcast(F32), 1.0)

                def mk_evict(hh, c_ps=c_ps, ctxr=ctxr, den_sb=den_sb):
                    def ev():
                        o = hh * HD
                        dr = hh * 32
                        for qh in range(2):
                            sl = slice(qh * 512, (qh + 1) * 512)
                            nc.scalar.copy(ctxr[o:o + HD, sl],
                                           c_ps[(hh, qh)][0:HD, :])
                            nc.scalar.copy(den_sb[dr:dr + 1, sl],
                                           c_ps[(hh, qh)][HD:HD + 1, :])
                    return ev

                if pc == 3:
                    mk_evict(0)()
                    mk_evict(1)()
                    emit_norm_psum(pc, ctxr, den_sb)
                else:
                    norm_stages = ([mk_evict(0), mk_evict(1)]
                                   + make_norm_stages(pc, ctxr, den_sb))

        # ---- output projection ----
        # p-major emission: all pair-0 matmuls first, so the PE only waits
        # on the last pair's normalize chain for the final 8 matmuls.
        ps_c.release()
        ps_s.release()
        ps_o = tc.alloc_tile_pool(name="ps_o", bufs=8, space="PSUM")
        with tc.tile_pool(name="outp", bufs=3) as out_p:
            o_ps = [ps_o.tile([P, D], F32, tag="ps_o", name=f"o_ps{st}")
                    for st in range(8)]
            for p_ in range(4):
                for st in range(8):
                    nc.tensor.matmul(o_ps[st][:],
                                     ctxn[p_][:, st * P:(st + 1) * P],
                                     wo_tl[:, p_ * D:(p_ + 1) * D],
                                     start=(p_ == 0), stop=(p_ == 3))
            for st in range(8):
                o_t = out_p.tile([P, D], F32, tag="out", name=f"out{st}")
                nc.vector.scalar_tensor_tensor(
                    o_t[:], o_ps[st][:], 1.0, wob_bc[:], ALU.bypass, ALU.add)
                q = nc.sync if st % 2 == 0 else nc.scalar
                q.dma_start(out_d.ap()[st * P:(st + 1) * P, :], o_t[:])
        ps_o.release()

    nc.compile()
    return nc


def to_bf16(x):
    return np.asarray(x, np.float32).astype(ml_dtypes.bfloat16)


def pack128(x):
    """[n*128, C] -> [128, n*C]: row-chunk c at columns [c*C, (c+1)*C)."""
    n = x.shape[0] // 128
    return np.ascontiguousarray(
        x.reshape(n, 128, x.shape[1]).transpose(1, 0, 2).reshape(128, -1))


def shard_inputs(u_enc, e_enc, logit_bpp, ue_mask, eu_mask,
                 wq_k, wq_b, wk_k, wk_b, wv_k, wv_b, wo_k, wo_b,
                 bpp_w, bpp_b):
    """Build the 8 per-core input maps (layout + bf16 rounding only)."""
    u_enc = np.asarray(u_enc, np.float32)
    e_enc = np.asarray(e_enc, np.float32)
    bpp = np.asarray(logit_bpp, np.float32)
    ue_m = np.asarray(ue_mask).astype(np.uint8)
    eu_m = np.asarray(eu_mask).astype(np.uint8)
    com = dict(
        wq=pack128(to_bf16(np.asarray(wq_k, np.float32).reshape(D, FH))),
        wk=pack128(to_bf16(np.asarray(wk_k, np.float32).reshape(D, FH))),
        wv=pack128(to_bf16(np.asarray(wv_k, np.float32).reshape(D, FH))),
        wo=pack128(to_bf16(np.asarray(wo_k, np.float32).reshape(FH, D))),
        wqb=np.asarray(wq_b, np.float32).reshape(FH).copy(),
        wkb=np.asarray(wk_b, np.float32).reshape(FH).copy(),
        wvb=np.asarray(wv_b, np.float32).reshape(FH).copy(),
        wob=np.asarray(wo_b, np.float32).reshape(D).copy(),
        bppw=np.asarray(bpp_w, np.float32).reshape(1, 1).copy(),
        bppb=np.asarray(bpp_b, np.float32).reshape(1, 1).copy(),
    )
    uT = [pack128(to_bf16(u_enc[b].T)) for b in range(B)]
    eT = [pack128(to_bf16(e_enc[b].T)) for b in range(B)]
    bpp_bf = pack128(to_bf16(bpp))
    bppT_bf = pack128(to_bf16(np.ascontiguousarray(bpp.T)))
    in_maps = []
    for i in range(N_CORES):
        d, b = divmod(i, B)
        if d == 0:      # u queries, e keys -> u_update[b]
            m = dict(encQT=uT[b], encKT=eT[b], bpp=bppT_bf,
                     mask=pack128(np.ascontiguousarray(ue_m[b, 0].T)))
        else:           # e queries, u keys -> e_update[b]
            m = dict(encQT=eT[b], encKT=uT[b], bpp=bpp_bf,
                     mask=pack128(np.ascontiguousarray(eu_m[b, 0].T)))
        m.update(com)
        in_maps.append(m)
    return in_maps


_NC = None


def kernel(**inputs):
    global _NC
    if _NC is None:
        _NC = build_module()
    in_maps = shard_inputs(**inputs)
    res = bass_utils.run_bass_kernel_spmd(
        _NC, in_maps, core_ids=list(range(N_CORES)))
    u_update = np.stack([res.results[b]["out"] for b in range(B)])
    e_update = np.stack([res.results[B + b]["out"] for b in range(B)])
    return u_update, e_update


if __name__ == "__main__":
    # single-core CoreSim check of one (direction, batch) unit
    from concourse.bass_interp import CoreSim

    rng = np.random.default_rng(0)
    u = rng.standard_normal((B, L, D)).astype(np.float32)
    e = rng.standard_normal((B, L, D)).astype(np.float32)
    bpp = rng.standard_normal((L, L)).astype(np.float32)
    uem = (rng.random((B, 1, L, L)) < 0.9)
    eum = (rng.random((B, 1, L, L)) < 0.9)
    w = 1.0 / np.sqrt(D)
    wq = (rng.standard_normal((D, H, HD)) * w).astype(np.float32)
    wk = (rng.standard_normal((D, H, HD)) * w).astype(np.float32)
    wv = (rng.standard_normal((D, H, HD)) * w).astype(np.float32)
    wo = (rng.standard_normal((H, HD, D)) / np.sqrt(FH)).astype(np.float32)
    zq = (rng.standard_normal((H, HD)) * 0.1).astype(np.float32)
    zo = (rng.standard_normal((D,)) * 0.1).astype(np.float32)

    nc = build_module()
    in_maps = shard_inputs(u, e, bpp, uem, eum, wq, zq, wk, zq, wv, zq,
                           wo, zo, np.float32(1.3), np.float32(-0.2))

    core = 0
    sim = CoreSim(nc, trace=False)
    for k, vv in in_maps[core].items():
        sim.tensor(k)[:] = vv
    sim.simulate(check_with_hw=False)
    got = np.array(sim.tensor("out"))

    def ref_unit(encQ, encK, bias_qk, mask_qk):
        q = SCALE * (encQ @ wq.reshape(D, FH) + zq.reshape(FH))
        kk = encK @ wk.reshape(D, FH) + zq.reshape(FH)
        vv = encK @ wv.reshape(D, FH) + zq.reshape(FH)
        accum = np.zeros((L, D), np.float64)
        for h in range(H):
            qi = q[:, h * HD:(h + 1) * HD]
            ki = kk[:, h * HD:(h + 1) * HD]
            vi = vv[:, h * HD:(h + 1) * HD]
            s = qi @ ki.T + bias_qk
            s = np.where(mask_qk, s, -np.inf)
            s = s - s.max(-1, keepdims=True)
            p_ = np.exp(s)
            p_ /= p_.sum(-1, keepdims=True)
            accum += (p_ @ vi) @ wo[h]
        return (accum + zo).astype(np.float32)

    bq = 1.3 * bpp + -0.2
    exp_out = ref_unit(u[0], e[0], bq, uem[0, 0])
    err = np.abs(got - exp_out).max() / np.abs(exp_out).max()
    print("unit relerr vs numpy:", err)


# revision 31
# speedup vs baseline: 1.0485x; 1.0485x over previous
"""Trainium2 Bass kernel: MultiHeadCrossAttentionWithBias.

Reference computation (per batch b):
  q_u = scale*(u_enc @ wq + wq_b); k/v from e_enc (and vice versa)
  ue_w = softmax(q_u k_e^T + bpp + mask*-inf); u_ctx = ue_w @ v_e
  u_update = u_ctx @ wo + wo_b                     (same mirrored for e)

Sharding: the problem decomposes into 8 fully independent attention units:
(batch b, direction d) for b in 0..3, d in {u->e, e->u}. Core i = (d, b)
handles one unit end-to-end; no collectives needed.

Host prep is layout/precision only (transposes, slices, fp32->bf16
rounding of matmul operands); all FLOPs run on device.

Per-core inputs:
  encQT  [D=512, L=1024] bf16  query-side encoder, transposed
  encKT  [D=512, L=1024] bf16  key-side encoder, transposed
  bpp    [L, L] bf16           logit bias oriented [k, q]
  mask   [L, L] uint8          mask oriented [k, q]
  wq/wk/wv [D, 512] bf16, wo [512, D] bf16, biases f32

On-device math (per core), all matmul operands bf16 (FWL-friendly, no
fp32 slow paths; PSUM accumulation stays f32):
  qT[f, s] = scale*(wq^T encQT + wq_b)   (f = h*64+hd on partitions)
  kT[f, s] =        wk^T encKT + wk_b
  v[s, f]  =        encKT^T wv + wv_b    (+ fused ones column per head)
  CB[k, q] = bpp_w*bpp + bpp_b + (mask-1)*1e30   (DVE+gpsimd, -> bf16)
  per head h, k-chunk kc:
      S^T = CB[kc]                 (PE: identity-stationary matmul, start)
      S^T += kT_h^T qT_h           (PE accumulation, stop)
      E = exp(S^T)                 (ACT; no max-subtraction: logits O(10))
      [ctx^T; den] += [v_h | 1]^T E  (PE, PSUM accumulation over kc)
  The CB injection rides the PE (instead of a DVE add on the critical
  path) so the tensor engine stream stays dense: PE-HAM then holds the
  2.4 GHz clock state through the attention phase.
  rcp = approx_reciprocal(den); partition-broadcast via PE matmul with a
  2-row selector stationary (no DRAM bounce)
  ctxn[pair] = ctx^T * rcp  (DVE, odd head written to partitions 64..127)
  out[s, e] = sum_pair ctxn_p^T wo_p + wo_b   (PE + DVE bias-add eviction)
"""

import numpy as np
from contextlib import ExitStack

import ml_dtypes

import concourse.bass as bass
import concourse.tile as tile
import concourse.bacc as bacc
import concourse.mybir as mybir
from concourse.masks import make_identity
from concourse import bass_utils

F32 = mybir.dt.float32
U8 = mybir.dt.uint8
BF16 = mybir.dt.bfloat16
AF = mybir.ActivationFunctionType
ALU = mybir.AluOpType

B, L, D, H, HD = 4, 1024, 512, 8, 64
P = 128
FH = H * HD            # 512
SCALE = 1.0 / np.sqrt(HD)
NEG = -1.0e30
N_CORES = 8


def bcast_ap(dram_ap, parts):
    """Partition-step-0 broadcast AP over a DRAM row."""
    return bass.AP(tensor=dram_ap.tensor, offset=dram_ap.offset,
                   ap=[[0, parts]] + list(dram_ap.ap))


def build_module():
    nc = bacc.Bacc("TRN2", target_bir_lowering=False, debug=False)

    # inputs packed on host to [128, n*C]: row-chunk c of the logical
    # tensor sits at columns [c*C, (c+1)*C) -> one or two DMA
    # descriptors per tensor instead of one per 128-row chunk
    encQT_d = nc.dram_tensor("encQT", [P, 4 * L], BF16, kind="ExternalInput")
    encKT_d = nc.dram_tensor("encKT", [P, 4 * L], BF16, kind="ExternalInput")
    wq_d = nc.dram_tensor("wq", [P, 4 * FH], BF16, kind="ExternalInput")
    wk_d = nc.dram_tensor("wk", [P, 4 * FH], BF16, kind="ExternalInput")
    wv_d = nc.dram_tensor("wv", [P, 4 * FH], BF16, kind="ExternalInput")
    wo_d = nc.dram_tensor("wo", [P, 4 * D], BF16, kind="ExternalInput")
    bpp_d = nc.dram_tensor("bpp", [P, 8 * L], BF16, kind="ExternalInput")
    mask_d = nc.dram_tensor("mask", [P, 8 * L], U8, kind="ExternalInput")
    wqb_d = nc.dram_tensor("wqb", [FH], F32, kind="ExternalInput")
    wkb_d = nc.dram_tensor("wkb", [FH], F32, kind="ExternalInput")
    wvb_d = nc.dram_tensor("wvb", [FH], F32, kind="ExternalInput")
    wob_d = nc.dram_tensor("wob", [D], F32, kind="ExternalInput")
    bppw_d = nc.dram_tensor("bppw", [1, 1], F32, kind="ExternalInput")
    bppb_d = nc.dram_tensor("bppb", [1, 1], F32, kind="ExternalInput")
    out_d = nc.dram_tensor("out", [L, D], F32, kind="ExternalOutput")

    with tile.TileContext(nc) as tc, ExitStack() as ctx:
        const = ctx.enter_context(tc.tile_pool(name="const", bufs=1))
        qkT_p = ctx.enter_context(tc.tile_pool(name="qkT", bufs=8))
        v_p = ctx.enter_context(tc.tile_pool(name="v", bufs=8))
        wo_p = ctx.enter_context(tc.tile_pool(name="wo", bufs=1))
        cb_p = ctx.enter_context(tc.tile_pool(name="cb", bufs=8))
        ps_s = tc.alloc_tile_pool(name="ps_s", bufs=2, space="PSUM")
        ps_c = tc.alloc_tile_pool(name="ps_c", bufs=4, space="PSUM")

        # ---- small bias prep (tiny DMAs) ----
        # bpp_w / bpp_b broadcast to [128,1] columns
        bw_col = const.tile([P, 1], F32)
        nc.gpsimd.dma_start(bw_col[:], bcast_ap(bppw_d.ap()[0:1, :], P))
        bb_col = const.tile([P, 1], F32)
        nc.gpsimd.dma_start(bb_col[:], bcast_ap(bppb_d.ap()[0:1, :], P))
        # projection biases
        wqb_raw = const.tile([P, 4], F32)
        nc.gpsimd.dma_start(wqb_raw[:], wqb_d.ap().rearrange("(c p) -> p c", p=P))
        wqb_sc = const.tile([P, 4], F32)
        nc.vector.tensor_scalar_mul(wqb_sc[:], wqb_raw[:], float(SCALE))
        wkb_c = const.tile([P, 4], F32)
        nc.gpsimd.dma_start(wkb_c[:], wkb_d.ap().rearrange("(c p) -> p c", p=P))
        wvb_bc = const.tile([P, FH], F32)
        nc.gpsimd.dma_start(wvb_bc[:], bcast_ap(wvb_d.ap(), P))
        wob_bc = const.tile([P, D], F32)
        nc.gpsimd.dma_start(wob_bc[:], bcast_ap(wob_d.ap(), P))

        # selector for the denominator partition-broadcast (engine writes
        # must start at partition 0/32/64/96, so the two live rows sit at
        # partitions 0 and 32): sel[0, 0:64] = 1, sel[32, 64:128] = 1.
        # f32r keeps the denominator at full precision through the PE.
        F32R = mybir.dt.float32r
        sel = const.tile([33, P], F32R)
        nc.gpsimd.memset(sel[:].bitcast(F32), 0.0)
        nc.gpsimd.memset(sel[0:1, 0:HD].bitcast(F32), 1.0)
        nc.gpsimd.memset(sel[32:33, HD:P].bitcast(F32), 1.0)

        # ---- bias factor ECB[k, q] = exp(bpp*w + b + (mask-1)*1e30) ----
        # softmax bias applied multiplicatively: exp(S + CB) =
        # exp(S) * exp(CB), with exp(CB) precomputed overlapped with the
        # projections. Masked entries become exact 0, so the post-softmax
        # re-mask is free. (m*1e30) + (-1e30) is exact for m in {0,1}.
        # The mask/bpp DMAs ride the gpsimd queue concurrently with the
        # sync-queue weight/encoder loads, landing in a deep dedicated
        # pool so the shared DMA engines never block on tile reuse. The
        # compute (DVE scale ops, pool add, ACT exp) is emitted inside
        # the q-projection loop, two k-chunks per pc, so no engine sees
        # a burst right when attention starts.
        cb = []
        cbd_p = tc.alloc_tile_pool(name="cbdma", bufs=1)
        cbt_p = tc.alloc_tile_pool(name="cbtmp", bufs=3)
        mask_tl = cbd_p.tile([P, 8 * L], U8, tag="m", name="mask")
        bpp_tl = cbd_p.tile([P, 8 * L], BF16, tag="b", name="bpp")

        for i in range(4):
            s = slice(i * 2 * L, (i + 1) * 2 * L)
            nc.gpsimd.dma_start(mask_tl[:, s], mask_d.ap()[:, s])
        for i in range(8):
            s = slice(i * L, (i + 1) * L)
            nc.gpsimd.dma_start(bpp_tl[:, s], bpp_d.ap()[:, s])

        def emit_cb_compute(kc):
            mn_t = cbt_p.tile([P, L], BF16, tag="mn", name=f"mn{kc}")
            nc.vector.tensor_scalar(mn_t[:],
                                    mask_tl[:, kc * L:(kc + 1) * L], -NEG,
                                    NEG, ALU.mult, ALU.add)
            bs_t = cbt_p.tile([P, L], BF16, tag="bs", name=f"bs{kc}")
            nc.vector.tensor_scalar(bs_t[:],
                                    bpp_tl[:, kc * L:(kc + 1) * L],
                                    bw_col[:, 0:1], bb_col[:, 0:1],
                                    ALU.mult, ALU.add)
            c_t = cbt_p.tile([P, L], F32, tag="cb", name=f"cb{kc}")
            nc.gpsimd.tensor_add(c_t[:], mn_t[:], bs_t[:])
            e_t = cb_p.tile([P, L], BF16, tag="ecb", name=f"ecb{kc}")
            nc.scalar.activation(e_t[:], c_t[:], AF.Exp)
            cb.append(e_t)

        # ---- projections ----
        qT, kT, v_aug = [], [], []
        with tc.tile_pool(name="enc", bufs=1) as enc_p, \
             tc.tile_pool(name="wqkv", bufs=1) as w_p:
            # ~4 descriptors per tensor: parallel across DMA engines
            # but cheap to issue; weight and encoder issue ride separate
            # queues (sync / vector) so neither serializes the other
            def loadn(pool, eng, dram, cols, n, tag, dt=BF16):
                t = pool.tile([P, cols], dt, tag=tag,
                              name=f"t_{dram.name}")
                step = cols // n
                for i in range(n):
                    eng.dma_start(t[:, i * step:(i + 1) * step],
                                  dram.ap()[:, i * step:(i + 1) * step])
                return t

            wq_tl = loadn(w_p, nc.sync, wq_d, 4 * FH, 4, "wq")
            eq_tl = loadn(enc_p, nc.scalar, encQT_d, 4 * L, 4, "eq")
            wk_tl = loadn(w_p, nc.sync, wk_d, 4 * FH, 4, "wk")
            ek_tl = loadn(enc_p, nc.scalar, encKT_d, 4 * L, 4, "ek")
            wv_tl = loadn(w_p, nc.sync, wv_d, 4 * FH, 4, "wv")
            wq_t = [wq_tl[:, dc * FH:(dc + 1) * FH] for dc in range(4)]
            wk_t = [wk_tl[:, dc * FH:(dc + 1) * FH] for dc in range(4)]
            wv_t = [wv_tl[:, dc * FH:(dc + 1) * FH] for dc in range(4)]
            eq = [eq_tl[:, dc * L:(dc + 1) * L] for dc in range(4)]
            ek = [ek_tl[:, dc * L:(dc + 1) * L] for dc in range(4)]


            # qT / kT: [f, s] packed two heads per 128-partition chunk
            for which, w_tl, enc_tl, out_list in (
                ("q", wq_tl, eq_tl, qT), ("k", wk_tl, ek_tl, kT),
            ):
                for pc in range(4):
                    o = qkT_p.tile([P, L], BF16, tag="qkT",
                                   name=f"{which}T{pc}")
                    for sh in range(2):
                        ps = ps_s.tile([P, 512], F32, tag="ps_s",
                                       name=f"ps_{which}{pc}_{sh}")
                        for dc in range(4):
                            nc.tensor.matmul(
                                ps[:],
                                w_tl[:, dc * FH + pc * P:
                                     dc * FH + (pc + 1) * P],
                                enc_tl[:, dc * L + sh * 512:
                                       dc * L + (sh + 1) * 512],
                                start=(dc == 0), stop=(dc == 3))
                        sl = slice(sh * 512, (sh + 1) * 512)
                        # evict on DVE: the ACT queue stays free for the
                        # ecb exps + attention exps
                        if which == "q":
                            nc.vector.tensor_scalar(
                                o[:, sl], ps[:], float(SCALE),
                                wqb_sc[:, pc:pc + 1], ALU.mult, ALU.add)
                        else:
                            nc.vector.tensor_scalar(
                                o[:, sl], ps[:], wkb_c[:, pc:pc + 1], None,
                                ALU.add)
                    out_list.append(o)
                    emit_cb_compute(pc if which == "q" else 4 + pc)

            # v: [s, f] with ones column interleaved per head ([128, 8*65])
            for sc in range(8):
                ps = ps_s.tile([P, 512], F32, tag="ps_s", name=f"ps_v{sc}")
                for dc in range(4):
                    nc.tensor.matmul(
                        ps[:],
                        ek_tl[:, dc * L + sc * P:dc * L + sc * P + P],
                        wv_tl[:, dc * FH:(dc + 1) * FH],
                        start=(dc == 0), stop=(dc == 3))
                va = v_p.tile([P, H * (HD + 1)], BF16, tag="v", name=f"v{sc}")
                vg = va[:].rearrange("p (h c) -> p h c", c=HD + 1)
                nc.vector.scalar_tensor_tensor(
                    vg[:, :, 0:HD],
                    ps[:].rearrange("p (h c) -> p h c", c=HD), 1.0,
                    wvb_bc[:].rearrange("p (h c) -> p h c", c=HD),
                    ALU.bypass, ALU.add)
                nc.vector.memset(vg[:, :, HD:HD + 1], 1.0)
                v_aug.append(va)

        cbt_p.release()
        cbd_p.release()

        # ---- wo load (late: not projection-critical) ----
        wo_tl = wo_p.tile([P, 4 * D], BF16, tag="wo", name="wo")
        for i in range(4):
            s = slice(i * D, (i + 1) * D)
            nc.gpsimd.dma_start(wo_tl[:, s], wo_d.ap()[:, s])

        # ---- attention (head pairs, PE row-tiled QK) ----
        # The two heads of a pair occupy partitions 0..63 / 64..127 of
        # qT/kT, so their K=64 QK matmuls land in different PE row
        # groups and execute concurrently (row tiling: ~2x QK
        # throughput). Per (pair, kc): 4 QK matmuls in ~2 matmul-times
        # -> exp per head (ACT) -> bf16 multiply by exp(CB) (DVE) ->
        # lagged PV (full-K contraction, serial). The pair softmax
        # denominators are broadcast via a DRAM bounce (no PSUM), with
        # the DVE/pool work spread one stage per kc of the next pair;
        # the last pair broadcasts through the PE into freed PSUM.
        ctxn_p = ctx.enter_context(tc.tile_pool(name="ctxn", bufs=4))
        ctxr_p = ctx.enter_context(tc.tile_pool(name="ctxr", bufs=2))
        den_p = ctx.enter_context(tc.tile_pool(name="den", bufs=3))
        den_d = nc.dram_tensor("den_scratch", [H, L], F32, kind="Internal")
        ctxn = [None] * 4

        def make_norm_stages(pc, ctxr, den_sb):
            # DRAM-bounce partition-broadcast of the raw denominators,
            # then reciprocal (DVE) + multiply (gpsimd), staged so no
            # engine sees a lump right when the next pair starts.
            rb = den_p.tile([P, L], F32, tag="rb", name=f"rb{pc}")
            rcp_sb = den_p.tile([P, L], F32, tag="rcp", name=f"rcp{pc}")

            def st_a():
                nc.sync.dma_start(den_d.ap()[2 * pc:2 * pc + 1, :],
                                  den_sb[0:1, :].bitcast(F32))
                nc.sync.dma_start(den_d.ap()[2 * pc + 1:2 * pc + 2, :],
                                  den_sb[32:33, :].bitcast(F32))
                nc.gpsimd.dma_start(
                    rb[0:HD, :], bcast_ap(den_d.ap()[2 * pc:2 * pc + 1, :], HD))
                nc.gpsimd.dma_start(
                    rb[HD:P, :],
                    bcast_ap(den_d.ap()[2 * pc + 1:2 * pc + 2, :], HD))

            def st_rcp(qh):
                sl = slice(qh * 512, (qh + 1) * 512)
                nc.vector.reciprocal(rcp_sb[:, sl], rb[:, sl])

            def st_mul(qh):
                sl = slice(qh * 512, (qh + 1) * 512)
                nc.gpsimd.tensor_mul(ctxn[pc][:, sl], ctxr[:, sl],
                                     rcp_sb[:, sl])

            return [st_a, lambda: st_rcp(0), lambda: st_mul(0),
                    lambda: st_rcp(1), lambda: st_mul(1)]

        def emit_norm_psum(pc, ctxr, den_sb):
            # last pair: PSUM is free, broadcast through the PE
            for qh in range(2):
                sl = slice(qh * 512, (qh + 1) * 512)
                rb_ps = ps_c.tile([P, 512], F32, tag="ps_c",
                                  name=f"rbl_{pc}_{qh}")
                nc.tensor.matmul(rb_ps[:], sel[:], den_sb[:, sl],
                                 start=True, stop=True)
                rcp_sb = den_p.tile([P, 512], F32, tag="rcpl",
                                    name=f"rcpl_{pc}_{qh}")
                nc.vector.reciprocal(rcp_sb[:, 0:256], rb_ps[:, 0:256])
                nc.vector.reciprocal(rcp_sb[:, 256:512], rb_ps[:, 256:512])
                nc.gpsimd.tensor_mul(ctxn[pc][:, sl], ctxr[:, sl],
                                     rcp_sb[:])

        norm_stages = []
        with tc.tile_pool(name="e", bufs=14) as e_p, \
             tc.tile_pool(name="e0", bufs=4) as e0_p:
            for pc in range(4):
                LAG = 6 if pc == 0 else 3
                c_ps = {(hh, qh): ps_c.tile([HD + 1, 512], F32, tag="ps_c",
                                            name=f"c_ps_{pc}_{hh}_{qh}")
                        for hh in range(2) for qh in range(2)}
                e_ts = {}
                for kc in range(8 + LAG):
                    if kc < 8:
                        s_hh = [ps_s.tile([P, L], F32, tag="ps_s",
                                          name=f"s_ps_{pc}_{hh}_{kc}")
                                for hh in range(2)]
                        for qh in range(2):
                            sl = slice(qh * 512, (qh + 1) * 512)
                            for hh in range(2):
                                o = hh * HD
                                nc.tensor.matmul(
                                    s_hh[hh][:, sl],
                                    kT[pc][o:o + HD, kc * P:(kc + 1) * P],
                                    qT[pc][o:o + HD, sl],
                                    start=True, stop=True)
                        for hh in range(2):
                            e0 = e0_p.tile([P, L], BF16, tag="e0",
                                           name=f"e0_{pc}_{hh}_{kc}")
                            nc.scalar.activation(e0[:], s_hh[hh][:], AF.Exp)
                            et = e_p.tile([P, L], BF16, tag="e",
                                          name=f"e_{pc}_{hh}_{kc}")
                            nc.vector.tensor_mul(et[:], e0[:], cb[kc][:])
                            e_ts[(hh, kc)] = et
                    if norm_stages:
                        norm_stages.pop(0)()
                    if kc >= LAG:
                        kp = kc - LAG
                        for hh in range(2):
                            h = 2 * pc + hh
                            for qh in range(2):
                                sl = slice(qh * 512, (qh + 1) * 512)
                                nc.tensor.matmul(
                                    c_ps[(hh, qh)][:],
                                    v_aug[kp][:, h * (HD + 1):
                                              (h + 1) * (HD + 1)],
                                    e_ts[(hh, kp)][:, sl],
                                    start=(kp == 0), stop=(kp == 7))
                # evict raw ctx + denominator rows (ACT, odd head
                # partition-shifted to 64..127), freeing PSUM. For pairs
                # 0-2 the evictions are staged into the next pair's kc
                # loop so the ACT queue never lumps ahead of its exps.
                ctxn[pc] = ctxn_p.tile([P, L], BF16, tag="ctxn",
                                       name=f"ctxn{pc}")
                ctxr = ctxr_p.tile([P, L], F32, tag="ctxr",
                                   name=f"ctxr{pc}")
                den_sb = den_p.tile([33, L], mybir.dt.float32r,
                                    tag="den", name=f"den{pc}")
                # rows 1..31 are read (x0) by the last pair's broadcast
                nc.vector.memset(den_sb[:].bitcast(F32), 1.0)

                def mk_evict(hh, c_ps=c_ps, ctxr=ctxr, den_sb=den_sb):
                    def ev():
                        o = hh * HD
                        dr = hh * 32
                        for qh in range(2):
                            sl = slice(qh * 512, (qh + 1) * 512)
                            nc.scalar.copy(ctxr[o:o + HD, sl],
                                           c_ps[(hh, qh)][0:HD, :])
                            nc.scalar.copy(den_sb[dr:dr + 1, sl],
                                           c_ps[(hh, qh)][HD:HD + 1, :])
                    return ev

                if pc == 3:
                    mk_evict(0)()
                    mk_evict(1)()
                    emit_norm_psum(pc, ctxr, den_sb)
                else:
                    norm_stages = ([mk_evict(0), mk_evict(1)]
                                   + make_norm_stages(pc, ctxr, den_sb))

        # ---- output projection ----
        # p-major emission: all pair-0 matmuls first, so the PE only waits
        # on the last pair's normalize chain for the final 8 matmuls.
        ps_c.release()
        ps_s.release()
        ps_o = tc.alloc_tile_pool(name="ps_o", bufs=8, space="PSUM")
        with tc.tile_pool(name="outp", bufs=3) as out_p:
            o_ps = [ps_o.tile([P, D], F32, tag="ps_o", name=f"o_ps{st}")
                    for st in range(8)]
            for p_ in range(4):
                for st in range(8):
                    nc.tensor.matmul(o_ps[st][:],
                                     ctxn[p_][:, st * P:(st + 1) * P],
                                     wo_tl[:, p_ * D:(p_ + 1) * D],
                                     start=(p_ == 0), stop=(p_ == 3))
            for st in range(8):
                o_t = out_p.tile([P, D], F32, tag="out", name=f"out{st}")
                nc.vector.scalar_tensor_tensor(
                    o_t[:], o_ps[st][:], 1.0, wob_bc[:], ALU.bypass, ALU.add)
                nc.sync.dma_start(out_d.ap()[st * P:(st + 1) * P, :], o_t[:])
        ps_o.release()

    nc.compile()
    return nc


def to_bf16(x):
    return np.asarray(x, np.float32).astype(ml_dtypes.bfloat16)


def pack128(x):
    """[n*128, C] -> [128, n*C]: row-chunk c at columns [c*C, (c+1)*C)."""
    n = x.shape[0] // 128
    return np.ascontiguousarray(
        x.reshape(n, 128, x.shape[1]).transpose(1, 0, 2).reshape(128, -1))


def shard_inputs(u_enc, e_enc, logit_bpp, ue_mask, eu_mask,
                 wq_k, wq_b, wk_k, wk_b, wv_k, wv_b, wo_k, wo_b,
                 bpp_w, bpp_b):
    """Build the 8 per-core input maps (layout + bf16 rounding only)."""
    u_enc = np.asarray(u_enc, np.float32)
    e_enc = np.asarray(e_enc, np.float32)
    bpp = np.asarray(logit_bpp, np.float32)
    ue_m = np.asarray(ue_mask).astype(np.uint8)
    eu_m = np.asarray(eu_mask).astype(np.uint8)
    com = dict(
        wq=pack128(to_bf16(np.asarray(wq_k, np.float32).reshape(D, FH))),
        wk=pack128(to_bf16(np.asarray(wk_k, np.float32).reshape(D, FH))),
        wv=pack128(to_bf16(np.asarray(wv_k, np.float32).reshape(D, FH))),
        wo=pack128(to_bf16(np.asarray(wo_k, np.float32).reshape(FH, D))),
        wqb=np.asarray(wq_b, np.float32).reshape(FH).copy(),
        wkb=np.asarray(wk_b, np.float32).reshape(FH).copy(),
        wvb=np.asarray(wv_b, np.float32).reshape(FH).copy(),
        wob=np.asarray(wo_b, np.float32).reshape(D).copy(),
        bppw=np.asarray(bpp_w, np.float32).reshape(1, 1).copy(),
        bppb=np.asarray(bpp_b, np.float32).reshape(1, 1).copy(),
    )
    uT = [pack128(to_bf16(u_enc[b].T)) for b in range(B)]
    eT = [pack128(to_bf16(e_enc[b].T)) for b in range(B)]
    bpp_bf = pack128(to_bf16(bpp))
    bppT_bf = pack128(to_bf16(np.ascontiguousarray(bpp.T)))
    in_maps = []
    for i in range(N_CORES):
        d, b = divmod(i, B)
        if d == 0:      # u queries, e keys -> u_update[b]
            m = dict(encQT=uT[b], encKT=eT[b], bpp=bppT_bf,
                     mask=pack128(np.ascontiguousarray(ue_m[b, 0].T)))
        else:           # e queries, u keys -> e_update[b]
            m = dict(encQT=eT[b], encKT=uT[b], bpp=bpp_bf,
                     mask=pack128(np.ascontiguousarray(eu_m[b, 0].T)))
        m.update(com)
        in_maps.append(m)
    return in_maps


_NC = None


def kernel(**inputs):
    global _NC
    if _NC is None:
        _NC = build_module()
    in_maps = shard_inputs(**inputs)
    res = bass_utils.run_bass_kernel_spmd(
        _NC, in_maps, core_ids=list(range(N_CORES)))
    u_update = np.stack([res.results[b]["out"] for b in range(B)])
    e_update = np.stack([res.results[B + b]["out"] for b in range(B)])
    return u_update, e_update


if __name__ == "__main__":
    # single-core CoreSim check of one (direction, batch) unit
    from concourse.bass_interp import CoreSim

    rng = np.random.default_rng(0)
    u = rng.standard_normal((B, L, D)).astype(np.float32)
    e = rng.standard_normal((B, L, D)).astype(np.float32)
    bpp = rng.standard_normal((L, L)).astype(np.float32)
    uem = (rng.random((B, 1, L, L)) < 0.9)
    eum = (rng.random((B, 1, L, L)) < 0.9)
    w = 1.0 / np.sqrt(D)
    wq = (rng.standard_normal((D, H, HD)) * w).astype(np.float32)
    wk = (rng.standard_normal((D, H, HD)) * w).astype(np.float32)
    wv = (rng.standard_normal((D, H, HD)) * w).astype(np.float32)
    wo = (rng.standard_normal((H, HD, D)) / np.sqrt(FH)).astype(np.float32)
    zq = (rng.standard_normal((H, HD)) * 0.1).astype(np.float32)
    zo = (rng.standard_normal((D,)) * 0.1).astype(np.float32)

    nc = build_module()
    in_maps = shard_inputs(u, e, bpp, uem, eum, wq, zq, wk, zq, wv, zq,
                           wo, zo, np.float32(1.3), np.float32(-0.2))

    core = 0
    sim = CoreSim(nc, trace=False)
    for k, vv in in_maps[core].items():
        sim.tensor(k)[:] = vv
    sim.simulate(check_with_hw=False)
    got = np.array(sim.tensor("out"))

    def ref_unit(encQ, encK, bias_qk, mask_qk):
        q = SCALE * (encQ @ wq.reshape(D, FH) + zq.reshape(FH))
        kk = encK @ wk.reshape(D, FH) + zq.reshape(FH)
        vv = encK @ wv.reshape(D, FH) + zq.reshape(FH)
        accum = np.zeros((L, D), np.float64)
        for h in range(H):
            qi = q[:, h * HD:(h + 1) * HD]
            ki = kk[:, h * HD:(h + 1) * HD]
            vi = vv[:, h * HD:(h + 1) * HD]
            s = qi @ ki.T + bias_qk
            s = np.where(mask_qk, s, -np.inf)
            s = s - s.max(-1, keepdims=True)
            p_ = np.exp(s)
            p_ /= p_.sum(-1, keepdims=True)
            accum += (p_ @ vi) @ wo[h]
        return (accum + zo).astype(np.float32)

    bq = 1.3 * bpp + -0.2
    exp_out = ref_unit(u[0], e[0], bq, uem[0, 0])
    err = np.abs(got - exp_out).max() / np.abs(exp_out).max()
    print("unit relerr vs numpy:", err)
